# revision 2
# baseline (speedup 1.0000x reference)
"""CLAM-SB attention-MIL forward on 8 Trainium2 NeuronCores — fp8 DoubleRow version.

Per core (12544 patch rows; patch dim sharded over 8 cores):
  h2 = relu(h @ W1)            fp8 DoubleRow GEMMs (weights pre-scaled by 64)
  a = tanh(h2@Wa)              fp8 DR; sigmoid(x) computed as (tanh(x/2)+1)/2
  w = exp(2*(a*g)@Wattn/128)   fp8 DR matvec; exp fuses the softmax sum
  pooled += w * h2             DVE mult+reduce against a PE-broadcast weight row
  w -> nat32 score table       DVE 32x32 transpose

The PE stream is software-pipelined at depth 2 (W1 of pair i, gated-attention
GEMMs of pair i-1, attention matvec + pooling broadcast of pair i-2) so the
PE never waits on the activation chain and HAM stays warm.

fp8 scores only PRESELECT 64 top / 64 bottom candidate rows (two-level
per-partition max8; padded rows score exp(0)=1.0 and can never win either
side, so no masking is needed).  The 128 candidates are rescored exactly in
bf16 (gather + recompute h2/a/g/attn + instance logits); final top/bottom-8
ids are exact.  Two AllGathers: [exp-sum, pooled(512)] flies while the
candidate rescore runs; [16 x (score, li0, li1)] follows.  The global phase
is a handful of reductions + one 16x2 selection matmul.

All main-loop activations (relu/tanh/exp/copy) live in one ACT table set;
ln is prefetched during the collective.
"""

import sys

sys.path.insert(0, "/opt/trn_rl_repo")

import json

import ml_dtypes
import numpy as np

N = 100000
L = 1024
D1 = 512
D2 = 256
K = 8
NCLS = 2
NCORES = 8
RPC = 12800
COLS = RPC // 32  # 400
WS = 64.0
PAY1 = 1 + D1  # 513
PAY2 = 48


def _split_excess_waits(bir_bytes, max_waits=1):
    """Walrus accepts only ONE sync-wait per instruction; hoist extras onto
    same-engine NoOps placed immediately before."""
    d = json.loads(bir_bytes)
    for fn in d.get("functions", []):
        for blk in fn.get("blocks", []):
            out = []
            for ins in blk.get("instructions", []):
                si = ins.get("sync_info")
                waits = (si or {}).get("on_wait") or []
                if len(waits) > max_waits:
                    keep = waits[-max_waits:]
                    for i, w in enumerate(waits[:-max_waits]):
                        out.append(
                            {
                                "debug": ins.get("debug", 0),
                                "engine": ins["engine"],
                                "ins": [],
                                "outs": [],
                                "name": f"{ins['name']}-sw{i}",
                                "opcode": "NoOp",
                                "sync_info": {"on_update": [], "on_wait": [w]},
                                "text_hint": "waitsplit",
                            }
                        )
                    si["on_wait"] = keep
                out.append(ins)
            blk["instructions"] = out
    return json.dumps(d).encode()


_hook_installed = False


def _install_compile_hook():
    global _hook_installed
    if _hook_installed:
        return
    import concourse.bass2jax as b2j
    from concourse.bass_utils import compile_bir_kernel as _orig

    def _patched(bir_json, tmpdir, neff_name="file.neff"):
        return _orig(_split_excess_waits(bir_json), tmpdir, neff_name)

    b2j.compile_bir_kernel = _patched
    _hook_installed = True


def build():
    import concourse.bass as bass
    import concourse.mybir as mybir
    import concourse.tile as tile
    from concourse.masks import make_identity

    dt = mybir.dt
    AF = mybir.ActivationFunctionType
    OP = mybir.AluOpType
    DR = mybir.MatmulPerfMode.DoubleRow

    nc = bass.Bass()

    hsb = nc.dram_tensor("hsb", [RPC, L], dt.bfloat16, kind="ExternalInput")
    hst8 = nc.dram_tensor("hst8", [L, RPC], dt.float8e4, kind="ExternalInput")
    w1x = nc.dram_tensor("w1x", [L, D1], dt.float8e4, kind="ExternalInput")
    wax = nc.dram_tensor("wax", [D1, D2], dt.float8e4, kind="ExternalInput")
    wbx = nc.dram_tensor("wbx", [D1, D2], dt.float8e4, kind="ExternalInput")
    watx = nc.dram_tensor("watx", [D2, 16], dt.float8e4, kind="ExternalInput")
    w1b = nc.dram_tensor("w1b", [L, D1], dt.bfloat16, kind="ExternalInput")
    wab = nc.dram_tensor("wab", [D1, D2], dt.bfloat16, kind="ExternalInput")
    wbb = nc.dram_tensor("wbb", [D1, D2], dt.bfloat16, kind="ExternalInput")
    winb = nc.dram_tensor("winb", [D1, NCLS], dt.bfloat16, kind="ExternalInput")
    wattn = nc.dram_tensor("wattn", [D2, 1], dt.float32, kind="ExternalInput")
    wcls = nc.dram_tensor("wcls", [D1, NCLS], dt.float32, kind="ExternalInput")
    padcnt = nc.dram_tensor("padcnt", [1, 1], dt.float32, kind="ExternalInput")
    iotap = nc.dram_tensor("iotap", [32, 1], dt.float32, kind="ExternalInput")
    tgtm = nc.dram_tensor("tgtm", [16, 2], dt.float32, kind="ExternalInput")
    outd = nc.dram_tensor("out", [1, 3], dt.float32, kind="ExternalOutput")

    hst8r = hst8.rearrange("(lc p) r -> p lc r", p=128)

    halves = [512] * 25
    NH = len(halves)
    pairs = []
    i = 0
    while i < NH:
        if i + 1 < NH:
            pairs.append((i, halves[i], halves[i + 1]))
            i += 2
        else:
            pairs.append((i, halves[i], 0))
            i += 1

    with tile.TileContext(nc) as tc:
        with (
            tc.tile_pool(name="persist", bufs=1) as pp,
            tc.tile_pool(name="stream", bufs=3) as sp,
            tc.tile_pool(name="ppw", bufs=4, space="PSUM") as ppw,
            tc.tile_pool(name="dram", bufs=1, space="DRAM") as dp,
        ):
            payload1 = dp.tile([1, PAY1], dt.float32)
            gathered1 = dp.tile([NCORES, PAY1], dt.float32)
            payload2 = dp.tile([1, PAY2], dt.float32)
            gathered2 = dp.tile([NCORES, PAY2], dt.float32)
            warm_in = dp.tile([1, 1], dt.float32)
            warm_out = dp.tile([NCORES, 1], dt.float32)

            # ---- critical-path weights first (Sync DMA queue) ----
            w1_sb8 = pp.tile([128, 8, D1], dt.float8e4)
            nc.sync.dma_start(w1_sb8[:], w1x.rearrange("(ko p) n -> p ko n", p=128))
            wa_sb8 = pp.tile([128, 4, D2], dt.float8e4)
            nc.scalar.dma_start(wa_sb8[:], wax.rearrange("(ko p) n -> p ko n", p=128))
            wb_sb8 = pp.tile([128, 4, D2], dt.float8e4)
            nc.scalar.dma_start(wb_sb8[:], wbx.rearrange("(ko p) n -> p ko n", p=128))
            wat8 = pp.tile([128, 2, 16], dt.float8e4)
            nc.scalar.dma_start(wat8[:], watx.rearrange("(ko p) n -> p ko n", p=128))

            # small constants on the vector DMA queue
            padc_sb = pp.tile([1, 1], dt.float32)
            nc.scalar.dma_start(padc_sb[:], padcnt[:])
            iota_f = pp.tile([32, 1], dt.float32)
            nc.scalar.dma_start(iota_f[:], iotap[:])
            tgtm_sb = pp.tile([16, 2], dt.float32)
            nc.scalar.dma_start(tgtm_sb[:], tgtm[:])

            ident = pp.tile([128, 128], dt.float32)
            make_identity(nc, ident[:])
            identb = pp.tile([128, 128], dt.bfloat16)
            nc.vector.tensor_copy(identb[:], ident[:])
            ones16 = pp.tile([16, 1], dt.float32)
            nc.vector.memset(ones16[:], 1.0)
            onesr = pp.tile([1, 128], dt.float32)
            nc.vector.memset(onesr[:], 1.0)
            onesr_b = pp.tile([1, 128], dt.bfloat16)
            nc.vector.memset(onesr_b[:], 1.0)

            nat32 = pp.tile([32, COLS], dt.float32)
            s_parts = pp.tile([1, 32], dt.float32)
            nc.vector.memset(s_parts[:], 0.0)
            pacc = pp.tile([128, 4], dt.float32)
            nc.vector.memset(pacc[:], 0.0)
            psum_t = pp.tile([128, 4], dt.float32)

            # warm the collective path immediately (absorbs core start skew)
            warmsb = pp.tile([1, 1], dt.float32)
            nc.vector.memset(warmsb[:], 1.0)
            nc.scalar.dma_start(warm_in[:], warmsb[:])
            nc.gpsimd.collective_compute(
                "AllGather",
                mybir.AluOpType.bypass,
                replica_groups=[list(range(NCORES))],
                ins=[warm_in.opt()],
                outs=[warm_out.opt()],
            )

            # tail-only weights: emitted after the first pair, gpsimd DMA queue
            tailw = {}

            def emit_tail_weights():
                tailw["w1bf"] = pp.tile([128, 8, D1], dt.bfloat16, name="w1bf")
                nc.gpsimd.dma_start(
                    tailw["w1bf"][:], w1b.rearrange("(ko p) n -> p ko n", p=128)
                )
                tailw["wabf"] = pp.tile([128, 4, D2], dt.bfloat16, name="wabf")
                nc.gpsimd.dma_start(
                    tailw["wabf"][:], wab.rearrange("(ko p) n -> p ko n", p=128)
                )
                tailw["wbbf"] = pp.tile([128, 4, D2], dt.bfloat16, name="wbbf")
                nc.gpsimd.dma_start(
                    tailw["wbbf"][:], wbb.rearrange("(ko p) n -> p ko n", p=128)
                )
                tailw["winst_bf"] = pp.tile(
                    [128, 4, NCLS], dt.bfloat16, name="winst_bf"
                )
                nc.gpsimd.dma_start(
                    tailw["winst_bf"][:], winb.rearrange("(ko p) n -> p ko n", p=128)
                )
                tailw["wat_f"] = pp.tile([128, 2, 1], dt.float32, name="wat_f")
                nc.gpsimd.dma_start(
                    tailw["wat_f"][:], wattn.rearrange("(ko p) n -> p ko n", p=128)
                )
                tailw["wcls_sb"] = pp.tile([128, 4, NCLS], dt.float32, name="wcls_sb")
                nc.gpsimd.dma_start(
                    tailw["wcls_sb"][:], wcls.rearrange("(ko p) n -> p ko n", p=128)
                )
                tailw["wat_sb"] = pp.tile([128, 2, 1], dt.float32r, name="wat_sb")
                nc.scalar.activation(
                    tailw["wat_sb"][:, 0, :], tailw["wat_f"][:, 0, :], AF.Copy
                )
                nc.scalar.activation(
                    tailw["wat_sb"][:, 1, :], tailw["wat_f"][:, 1, :], AF.Copy
                )

            # ============ software-pipelined main loop (PE depth 2) ==========
            bgctx = tc.tile_pool(name="big", bufs=2, space="PSUM")
            bg = bgctx.__enter__()

            def stage_w1(hi, rA, rB):
                """DMA + W1 DR GEMMs + relu -> fp8 h2 for one pair."""
                r0 = hi * 512
                RR = rA + rB
                hs = [(0, rA)] + ([(rA, rB)] if rB else [])
                hT8 = sp.tile([128, 8, 1024], dt.float8e4, tag="hT", name="hT8")
                nc.sync.dma_start(hT8[:, :, :RR], hst8r[:, :, r0 : r0 + RR])
                h2b8 = sp.tile([128, 4, 1024], dt.float8e4, tag="h2", name="h2b8")
                for dc in range(4):
                    p1 = bg.tile([128, 1024], dt.float32, tag="ps_big", name="p1")
                    for kk in range(0, 8, 2):
                        for h0, hr in hs:
                            nc.tensor.matmul(
                                p1[:, h0 : h0 + hr],
                                lhsT=w1_sb8[:, kk : kk + 2, dc * 128 : (dc + 1) * 128],
                                rhs=hT8[:, kk : kk + 2, h0 : h0 + hr],
                                start=(kk == 0),
                                stop=(kk == 6),
                                perf_mode=DR,
                            )
                    nc.scalar.activation(
                        h2b8[:, dc, :RR], p1[:, :RR], AF.Relu, scale=1.0 / WS
                    )
                return {"hi": hi, "hs": hs, "RR": RR, "h2b8": h2b8}

            def stage_ag(st):
                """gated-attention DR GEMMs + tanh + fused sigmoid-mult."""
                RR, h2b8, hs = st["RR"], st["h2b8"], st["hs"]
                a_f = sp.tile([128, 2, 1024], dt.bfloat16, tag="a_f", name="a_f")
                g_f = sp.tile([128, 2, 1024], dt.bfloat16, tag="g_f", name="g_f")
                for wsb, scl, dst in (
                    (wa_sb8, 1.0 / WS, a_f),
                    (wb_sb8, 0.5 / WS, g_f),
                ):
                    for ec in range(2):
                        p2 = bg.tile([128, 1024], dt.float32, tag="ps_big", name="p2")
                        for kk in range(0, 4, 2):
                            for h0, hr in hs:
                                nc.tensor.matmul(
                                    p2[:, h0 : h0 + hr],
                                    lhsT=wsb[:, kk : kk + 2, ec * 128 : (ec + 1) * 128],
                                    rhs=h2b8[:, kk : kk + 2, h0 : h0 + hr],
                                    start=(kk == 0),
                                    stop=(kk == 2),
                                    perf_mode=DR,
                                )
                        nc.scalar.activation(
                            dst[:, ec, :RR], p2[:, :RR], AF.Tanh, scale=scl
                        )
                ag8 = sp.tile([128, 2, 1024], dt.float8e4, tag="ag", name="ag8")
                for ec in range(2):
                    nc.vector.scalar_tensor_tensor(
                        out=ag8[:, ec, :RR],
                        in0=g_f[:, ec, :RR],
                        scalar=1.0,
                        in1=a_f[:, ec, :RR],
                        op0=OP.add,
                        op1=OP.mult,
                    )
                st["ag8"] = ag8
                return st

            def stage_pool(st):
                """attn matvec, exp (+softmax sum), broadcast, pooling, scores."""
                hi, RR, h2b8, ag8 = st["hi"], st["RR"], st["h2b8"], st["ag8"]
                hs = st["hs"]
                w_row = sp.tile([1, 1024], dt.float32, tag="wrow", name="w_row")
                w_rowb = sp.tile([1, 1024], dt.bfloat16, tag="wrowb", name="w_rowb")
                junkb = sp.tile([128, 1024], dt.bfloat16, tag="junk", name="junkb")
                pwts = []
                for hh, (h0, hr) in enumerate(hs):
                    pwt = ppw.tile([128, 512], dt.float32, tag="pwb", name="pwt")
                    pwts.append(pwt)
                    nc.tensor.matmul(
                        pwt[0:1, :hr],
                        lhsT=wat8[:, :, 0:1],
                        rhs=ag8[:, :, h0 : h0 + hr],
                        start=True,
                        stop=True,
                        perf_mode=DR,
                    )
                for hh, (h0, hr) in enumerate(hs):
                    hidx = hi + hh
                    nc.scalar.activation(
                        w_row[0:1, h0 : h0 + hr],
                        pwts[hh][0:1, :hr],
                        AF.Exp,
                        scale=0.5 / WS,
                        accum_out=s_parts[:1, hidx : hidx + 1],
                    )
                    nc.scalar.activation(
                        w_rowb[0:1, h0 : h0 + hr],
                        pwts[hh][0:1, :hr],
                        AF.Exp,
                        scale=0.5 / WS,
                    )
                for hh, (h0, hr) in enumerate(hs):
                    nc.tensor.matmul(
                        pwts[hh][:, :hr],
                        lhsT=onesr_b[:1, :],
                        rhs=w_rowb[0:1, h0 : h0 + hr],
                        start=True,
                        stop=True,
                    )
                RRt = sum(hr for _, hr in hs)
                for dc in range(4):
                    for hh, (h0, hr) in enumerate(hs):
                        nc.vector.tensor_tensor(
                            junkb[:, h0 : h0 + hr],
                            h2b8[:, dc, h0 : h0 + hr],
                            pwts[hh][:, :hr],
                            op=OP.mult,
                        )
                    nc.vector.tensor_reduce(
                        psum_t[:, dc : dc + 1],
                        junkb[:, :RRt],
                        axis=mybir.AxisListType.X,
                        op=OP.add,
                    )
                nc.vector.tensor_tensor(pacc[:], pacc[:], psum_t[:], op=OP.add)
                for hh, (h0, hr) in enumerate(hs):
                    hidx = hi + hh
                    nc.sync.dma_start(
                        nat32[:32, hidx * 16 : hidx * 16 + hr // 32],
                        w_row[0:1, h0 : h0 + hr],
                    )

            inflight = []
            for pi, (hi, rA, rB) in enumerate(pairs):
                if len(inflight) >= 2:
                    stage_pool(inflight[-2])
                st = stage_w1(hi, rA, rB)
                inflight.append(st)
                if pi == 3:
                    emit_tail_weights()
                if len(inflight) >= 2:
                    stage_ag(inflight[-2])
            stage_pool(inflight[-2])
            stage_ag(inflight[-1])
            stage_pool(inflight[-1])

            bgctx.__exit__(None, None, None)
            tpctx = tc.tile_pool(name="tailp", bufs=1, space="PSUM")
            tp = tpctx.__enter__()

            wat_sb = tailw["wat_sb"]
            w1bf = tailw["w1bf"]
            wabf = tailw["wabf"]
            wbbf = tailw["wbbf"]
            winst_bf = tailw["winst_bf"]
            wcls_sb = tailw["wcls_sb"]

            # ---- softmax sum + pooled partials -> AllGather #1 (early) ----
            s_loc = pp.tile([1, 1], dt.float32)
            nc.vector.tensor_reduce(
                s_loc[:], s_parts[:1, :], axis=mybir.AxisListType.X, op=OP.add
            )
            nc.vector.tensor_tensor(s_loc[:], s_loc[:], padc_sb[:], op=OP.subtract)
            ppTw = tp.tile([128, 256], dt.float32, tag="tg", bufs=2, name="ppTw")
            ppT = ppTw[0:4, 0:128]
            nc.tensor.transpose(ppT, pacc[:], ident[:])
            paccT = pp.tile([4, 128], dt.float32)
            nc.vector.tensor_copy(paccT[:], ppT)
            nc.sync.dma_start(payload1[0:1, 0:1], s_loc[:])
            nc.sync.dma_start(
                payload1[0:1, 1 : 1 + D1].rearrange("o (k p) -> (o k) p", k=4),
                paccT[:],
            )
            nc.gpsimd.collective_compute(
                "AllGather",
                mybir.AluOpType.bypass,
                replica_groups=[list(range(NCORES))],
                ins=[payload1.opt()],
                outs=[gathered1.opt()],
            )

            # ---- candidate selection (no masks: pads score exp(0)=1.0) ----
            vt1 = pp.tile([32, 8], dt.float32)
            it1 = pp.tile([32, 8], dt.uint32)
            nc.vector.max(out=vt1[:], in_=nat32[:])
            nc.vector.max_index(out=it1[:], in_max=vt1[:], in_values=nat32[:])
            botm = pp.tile([32, COLS], dt.float32)
            nc.vector.tensor_scalar(botm[:], nat32[:], -1.0, None, op0=OP.mult)
            vb1 = pp.tile([32, 8], dt.float32)
            ib1 = pp.tile([32, 8], dt.uint32)
            nc.vector.max(out=vb1[:], in_=botm[:])
            nc.vector.max_index(out=ib1[:], in_max=vb1[:], in_values=botm[:])

            iota16 = pp.tile([32, 1], dt.float32)
            nc.vector.tensor_scalar(iota16[:], iota_f[:], 16.0, None, op0=OP.mult)
            rt_t = pp.tile([32, 8], dt.float32)
            rt_b = pp.tile([32, 8], dt.float32)
            for srci, dstt in ((it1, rt_t), (ib1, rt_b)):
                # row = 512*(c>>4) + (c&15) + 16*p, computed exactly in uint32
                fs = sp.tile([32, 8], dt.uint32, tag="fs", name="fs")
                nc.vector.tensor_scalar(
                    fs[:], srci[:], 4, 9,
                    op0=OP.logical_shift_right, op1=OP.logical_shift_left,
                )
                clu = sp.tile([32, 8], dt.uint32, tag="clu", name="clu")
                nc.vector.tensor_scalar(clu[:], srci[:], 15, None, op0=OP.bitwise_and)
                nc.vector.tensor_tensor(fs[:], fs[:], clu[:], op=OP.add)
                tmpf = sp.tile([32, 8], dt.float32, tag="tmpf", name="tmpf")
                nc.vector.tensor_copy(tmpf[:], fs[:])
                nc.vector.tensor_scalar(
                    dstt[:], tmpf[:], 1.0, iota16[:, 0:1], op0=OP.mult, op1=OP.add
                )

            # regroup [32,8]->[8,32] on four parallel DMA queues
            v8t8 = pp.tile([8, 32], dt.float32)
            nc.sync.dma_start(v8t8[:], vt1[:])
            r8t8 = pp.tile([8, 32], dt.float32)
            nc.gpsimd.dma_start(r8t8[:], rt_t[:])
            v8b8 = pp.tile([8, 32], dt.float32)
            nc.scalar.dma_start(v8b8[:], vb1[:])
            r8b8 = pp.tile([8, 32], dt.float32)
            nc.sync.dma_start(r8b8[:], rt_b[:])

            rowsf = pp.tile([128, 1], dt.float32)
            for half, (v8, r8) in enumerate(((v8t8, r8t8), (v8b8, r8b8))):
                w8 = pp.tile([8, 8], dt.float32, name=f"w8_{half}")
                nc.vector.max(out=w8[:], in_=v8[:])
                eq3 = sp.tile([8, 8, 32], dt.float32, tag="eq3", name="eq3")
                nc.vector.tensor_tensor(
                    eq3[:],
                    w8[:].unsqueeze(2).to_broadcast([8, 8, 32]),
                    v8[:].unsqueeze(1).to_broadcast([8, 8, 32]),
                    op=OP.is_equal,
                )
                nc.vector.tensor_tensor(
                    eq3[:],
                    eq3[:],
                    r8[:].unsqueeze(1).to_broadcast([8, 8, 32]),
                    op=OP.mult,
                )
                rowid = pp.tile([8, 8], dt.float32, name=f"rowid_{half}")
                nc.vector.tensor_reduce(
                    rowid[:], eq3[:], axis=mybir.AxisListType.X, op=OP.add
                )
                if half == 0:
                    nc.sync.dma_start(rowsf[0:64, 0:1], rowid[:])
                else:
                    nc.scalar.dma_start(rowsf[64:128, 0:1], rowid[:])
            rows_u = pp.tile([128, 1], dt.uint32)
            nc.vector.tensor_copy(rows_u[:], rowsf[:])

            # exact bf16 rescore of the 128 candidates
            hcand = pp.tile([128, L], dt.bfloat16)
            nc.gpsimd.indirect_dma_start(
                out=hcand[:],
                out_offset=None,
                in_=hsb[:, :],
                in_offset=bass.IndirectOffsetOnAxis(ap=rows_u[:, 0:1], axis=0),
            )
            hcT = pp.tile([128, 8, 128], dt.bfloat16)
            for lc in range(8):
                pctw = tp.tile([128, 256], dt.bfloat16, tag="tb", bufs=2, name="pctw")
                pct = pctw[:, 0:128]
                nc.tensor.transpose(
                    pct, hcand[:, lc * 128 : (lc + 1) * 128], identb[:]
                )
                nc.vector.tensor_copy(hcT[:, lc, :], pct)
            h2cT = pp.tile([128, 4, 128], dt.bfloat16)
            for dc in range(4):
                pcw = tp.tile([128, 256], dt.float32, tag="tg", bufs=2, name="pcw")
                pc = pcw[:, 0:128]
                for lc in range(8):
                    nc.tensor.matmul(
                        pc,
                        lhsT=w1bf[:, lc, dc * 128 : (dc + 1) * 128],
                        rhs=hcT[:, lc, :],
                        start=(lc == 0),
                        stop=(lc == 7),
                    )
                nc.scalar.activation(h2cT[:, dc, :], pc, AF.Relu)

            acT = pp.tile([128, 2, 128], dt.float32)
            gcT = pp.tile([128, 2, 128], dt.float32)
            for wsb, scl, dst in ((wabf, 1.0, acT), (wbbf, 0.5, gcT)):
                pag = tp.tile([128, 256], dt.float32, tag="tg", bufs=2, name="pag")
                for ec in range(2):
                    for dc in range(4):
                        nc.tensor.matmul(
                            pag[:, ec * 128 : (ec + 1) * 128],
                            lhsT=wsb[:, dc, ec * 128 : (ec + 1) * 128],
                            rhs=h2cT[:, dc, :],
                            start=(dc == 0),
                            stop=(dc == 3),
                        )
                nc.scalar.activation(
                    dst[:].rearrange("p a b -> p (a b)"), pag[:], AF.Tanh, scale=scl
                )
            agr = pp.tile([128, 2, 128], dt.float32r)
            nc.vector.scalar_tensor_tensor(
                out=agr[:], in0=gcT[:], scalar=1.0, in1=acT[:],
                op0=OP.add, op1=OP.mult,
            )
            p3c = ppw.tile([128, 512], dt.float32, tag="pwb", name="p3c")
            for ec in range(2):
                nc.tensor.matmul(
                    p3c[0:1, 0:128],
                    lhsT=wat_sb[:, ec, :],
                    rhs=agr[:, ec, :],
                    start=(ec == 0),
                    stop=(ec == 1),
                )
            pli2w = tp.tile([128, 256], dt.float32, tag="tg", bufs=2, name="pli2w")
            pli2 = pli2w[0:2, 0:128]
            for dc in range(4):
                nc.tensor.matmul(
                    pli2,
                    lhsT=winst_bf[:, dc, :],
                    rhs=h2cT[:, dc, :],
                    start=(dc == 0),
                    stop=(dc == 3),
                )
            stackc = pp.tile([3, 128], dt.float32)
            nc.scalar.activation(stackc[0:1, :], p3c[0:1, 0:128], AF.Copy)
            li2 = pp.tile([2, 128], dt.float32)
            nc.vector.tensor_copy(li2[:], pli2)
            nc.sync.dma_start(stackc[1:3, :], li2[:])
            pct3w = tp.tile([128, 256], dt.float32, tag="tg", bufs=2, name="pct3w")
            pct3 = pct3w[:, 0:3]
            nc.tensor.transpose(pct3, stackc[:], ident[0:3, 0:3])
            candT = pp.tile([128, 3], dt.float32)
            nc.vector.tensor_copy(candT[:], pct3)

            v16 = pp.tile([1, 16], dt.float32)
            nc.vector.max(out=v16[:1, 0:8], in_=stackc[0:1, 0:64])
            negb = pp.tile([1, 64], dt.float32)
            nc.vector.tensor_scalar(
                negb[:], stackc[0:1, 64:128], -1.0, None, op0=OP.mult
            )
            nb8 = pp.tile([1, 8], dt.float32)
            nc.vector.max(out=nb8[:], in_=negb[:])
            nc.vector.tensor_scalar(v16[:1, 8:16], nb8[:], -1.0, None, op0=OP.mult)

            pgb = ppw.tile([128, 512], dt.float32, tag="pwb", name="pgb")
            nc.tensor.matmul(
                pgb[:, 0:16], lhsT=onesr[:1, :], rhs=v16[:1, :], start=True, stop=True
            )
            S = pp.tile([128, 16], dt.float32)
            nc.vector.tensor_tensor(
                S[:], candT[:, 0:1].to_broadcast([128, 16]), pgb[:, 0:16],
                op=OP.is_equal,
            )
            pli16w = tp.tile([128, 256], dt.float32, tag="tg", bufs=2, name="pli16w")
            pli16 = pli16w[0:16, 0:2]
            nc.tensor.matmul(
                pli16, lhsT=S[:], rhs=candT[:, 1:3], start=True, stop=True
            )
            li16 = pp.tile([16, 2], dt.float32)
            nc.vector.tensor_copy(li16[:], pli16)

            trip = payload2[0:1, 0:PAY2].rearrange("o (w s) -> (o w) s", s=3)
            nc.sync.dma_start(trip[:, 0:1], v16[:1, :])
            nc.scalar.dma_start(trip[:, 1:3], li16[:])

            # prefetch the ln/exp table set during the collective
            lnscr = pp.tile([1, 1], dt.float32)
            nc.scalar.activation(lnscr[:], v16[0:1, 0:1], AF.Abs)
            nc.scalar.activation(lnscr[:], lnscr[:], AF.Ln)

            nc.gpsimd.collective_compute(
                "AllGather",
                mybir.AluOpType.bypass,
                replica_groups=[list(range(NCORES))],
                ins=[payload2.opt()],
                outs=[gathered2.opt()],
            )

            # ---- global phase ----
            zp = pp.tile([1, NCORES], dt.float32)
            nc.sync.dma_start(zp[:], gathered1[:, 0:1])
            Z = pp.tile([1, 1], dt.float32)
            nc.vector.tensor_reduce(Z[:], zp[:], axis=mybir.AxisListType.X, op=OP.add)
            Zr = pp.tile([1, 1], dt.float32)
            nc.vector.reciprocal(Zr[:], Z[:])

            pT4 = pp.tile([128, 4, NCORES], dt.float32)
            for k in range(4):
                qeng = (nc.sync, nc.scalar, nc.gpsimd, nc.sync)[k]
                qeng.dma_start(
                    pT4[:, k, :],
                    gathered1[:, 1 + k * 128 : 1 + (k + 1) * 128].rearrange(
                        "c p -> p c"
                    ),
                )
            MT4 = pp.tile([128, 4], dt.float32)
            nc.vector.tensor_reduce(
                MT4[:], pT4[:], axis=mybir.AxisListType.X, op=OP.add
            )
            pbag = ppw.tile([128, 512], dt.float32, tag="pwb", name="pbag")
            for k in range(4):
                nc.tensor.matmul(
                    pbag[0:1, 0:NCLS],
                    lhsT=MT4[:, k : k + 1],
                    rhs=wcls_sb[:, k, :],
                    start=(k == 0),
                    stop=(k == 3),
                )
            bag = pp.tile([1, NCLS], dt.float32)
            nc.vector.tensor_copy(bag[:], pbag[0:1, 0:NCLS])
            nc.vector.tensor_scalar(bag[:], bag[:], Zr[:1, 0:1], None, op0=OP.mult)

            HV3 = pp.tile([128, 3], dt.float32)
            nc.sync.dma_start(
                HV3[:], gathered2[:, 0:PAY2].rearrange("c (w s) -> c w s", s=3)
            )
            gs = pp.tile([1, 128], dt.float32)
            nc.scalar.dma_start(
                gs[:],
                gathered2[:, 0:PAY2].rearrange("c (w s) -> c w s", s=3)[:, :, 0:1],
            )
            gs3 = gs[0:1, :].rearrange("o (c w) -> o c w", w=16)
            g16 = pp.tile([1, 16], dt.float32)
            nc.vector.max(out=g16[:1, 0:8], in_=gs3[:, :, 0:8])
            gneg = pp.tile([1, NCORES, 8], dt.float32)
            nc.vector.tensor_scalar(gneg[:], gs3[:, :, 8:16], -1.0, None, op0=OP.mult)
            gb8 = pp.tile([1, 8], dt.float32)
            nc.vector.max(out=gb8[:], in_=gneg[:])
            nc.vector.tensor_scalar(g16[:1, 8:16], gb8[:], -1.0, None, op0=OP.mult)

            pgb2 = ppw.tile([128, 512], dt.float32, tag="pwb", name="pgb2")
            nc.tensor.matmul(
                pgb2[:, 0:16], lhsT=onesr[:1, :], rhs=g16[:1, :], start=True, stop=True
            )
            S2 = pp.tile([128, 16], dt.float32)
            nc.vector.tensor_tensor(
                S2[:], HV3[:, 0:1].to_broadcast([128, 16]), pgb2[:, 0:16],
                op=OP.is_equal,
            )
            pligw = tp.tile([128, 256], dt.float32, tag="tg", bufs=2, name="pligw")
            plig = pligw[0:16, 0:2]
            nc.tensor.matmul(plig, lhsT=S2[:], rhs=HV3[:, 1:3], start=True, stop=True)

            ex = pp.tile([16, NCLS], dt.float32)
            se = pp.tile([16, 1], dt.float32)
            nc.scalar.activation(ex[:], plig, AF.Exp, accum_out=se[:])
            lse = pp.tile([16, 1], dt.float32)
            nc.scalar.activation(lse[:], se[:], AF.Ln)
            xsel = pp.tile([16, 2], dt.float32)
            nc.vector.tensor_tensor(xsel[:], plig, tgtm_sb[:], op=OP.mult)
            lv = pp.tile([16, 1], dt.float32)
            nc.vector.tensor_reduce(
                lv[:], xsel[:], axis=mybir.AxisListType.X, op=OP.add
            )
            nc.vector.tensor_tensor(lv[:], lv[:], lse[:], op=OP.subtract)
            plow = tp.tile([128, 256], dt.float32, tag="tg", bufs=2, name="plow")
            plo = plow[0:1, 0:1]
            nc.tensor.matmul(plo, lhsT=ones16[:], rhs=lv[:], start=True, stop=True)
            loss = pp.tile([1, 1], dt.float32)
            nc.scalar.activation(loss[:], plo, AF.Copy, scale=-1.0 / 16.0)

            osb = pp.tile([1, 3], dt.float32)
            nc.vector.tensor_copy(osb[:, 0:2], bag[:])
            nc.vector.tensor_copy(osb[:, 2:3], loss[:])
            nc.sync.dma_start(outd[:], osb[:])
            tpctx.__exit__(None, None, None)

    return nc


# ---------------------------------------------------------------------------
# host-side sharding
# ---------------------------------------------------------------------------
def make_in_maps(h, W1, Wa, Wb, Wattn, Wcls, Winst):
    f8 = ml_dtypes.float8_e4m3
    bf = ml_dtypes.bfloat16
    ntot = RPC * NCORES
    n = h.shape[0]
    hp = np.zeros((ntot, L), dtype=np.float32)
    hp[:n] = np.asarray(h, np.float32)
    shards = hp.reshape(NCORES, RPC, L)

    w1x = (np.asarray(W1, np.float32) * WS).astype(f8)
    wax = (np.asarray(Wa, np.float32) * WS).astype(f8)
    wbx = (np.asarray(Wb, np.float32) * WS).astype(f8)
    watx = np.zeros((D2, 16), np.float32)
    watx[:, 0] = np.asarray(Wattn, np.float32)[:, 0] * WS
    watx = watx.astype(f8)

    in_maps = []
    for c in range(NCORES):
        lo = c * RPC
        valid = min(max(n - lo, 0), RPC)
        in_maps.append(
            {
                "hsb": shards[c].astype(bf),
                "hst8": np.ascontiguousarray(shards[c].T).astype(f8),
                "w1x": w1x,
                "wax": wax,
                "wbx": wbx,
                "watx": watx,
                "w1b": np.asarray(W1, np.float32).astype(bf),
                "wab": np.asarray(Wa, np.float32).astype(bf),
                "wbb": np.asarray(Wb, np.float32).astype(bf),
                "winb": np.asarray(Winst, np.float32).astype(bf),
                "wattn": np.asarray(Wattn, np.float32),
                "wcls": np.asarray(Wcls, np.float32),
                "padcnt": np.array([[float(RPC - valid)]], np.float32),
                "iotap": np.arange(32, dtype=np.float32).reshape(32, 1),
                "tgtm": np.repeat(
                    np.array([[0.0, 1.0], [1.0, 0.0]], np.float32), 8, axis=0
                ),
            }
        )
    return in_maps


_cache = {}


def _get_nc():
    if "nc" not in _cache:
        _cache["nc"] = build()
    return _cache["nc"]


def kernel(h, W1, b1, Wa, ba, Wb, bb, Wattn, battn, Wcls, bcls, Winst, binst,
           trace=False):
    for name, b in (("b1", b1), ("ba", ba), ("bb", bb), ("battn", battn),
                    ("bcls", bcls), ("binst", binst)):
        if np.any(np.asarray(b) != 0):
            raise NotImplementedError(f"nonzero bias {name} not supported")
    _install_compile_hook()
    from concourse.bass_utils import run_bass_kernel_spmd

    nc = _get_nc()
    in_maps = make_in_maps(np.asarray(h, np.float32), W1, Wa, Wb, Wattn, Wcls, Winst)
    res = run_bass_kernel_spmd(nc, in_maps, list(range(NCORES)), trace=trace)
    out = np.asarray(res.results[0]["out"], np.float32).reshape(3)
    if trace:
        return out, res
    return out
